# revision 1
# baseline (speedup 1.0000x reference)
"""BigramAttn Trainium2 kernel (8-core SPMD, raw Bass).

Reference computation (per batch b):
  e[0]   = sum_k enc[0,k] * h[k]
  e[s]   = sum_k (enc[s-1,:] @ M)[k] * h[k] * enc[s,k]          (s >= 1)
  e[s]  += sum_{k<3} (h @ affect)[k] * emb[s,k]
  out    = softmax(e)                                            # over s

Sharding: data-parallel over batch B=32 across 8 cores (4 batches/core).

Per core (batch-major, transposed layout [H, S]; h pre-folded into per-batch
M_b = M @ diag(h_b) on the host):
  A_T[k, t]  = sum_j M_b[j,k] * encT[j, s0+t]        (PE fp32r, 16 MMs/step)
  P[k, t]    = A_T[k, t] * encT[k, s0+t+1]           (DVE tensor_tensor, x2)
  P[0:3,:]  += ha[k] * embT[k, t]                    (DVE stt, affect term)
  e[t]       = sum_kt ones^T @ P_kt                  (PE, 4 accumulating MMs)
  softmax over 4096 logits per batch, batched as [4, 4096].

Host pre-transposes the enc shard to [4, 512, 4096]: the PE contracts over
partitions so H must land on partitions; DMA-transpose is 2-byte-only on
trn2 and on-device transposing would double engine work. Bytes to HBM are
identical. All matmuls run float32r (full PE rate at N>=256; measured
end-to-end rel err ~2e-3). fp32r ISA rules: even moving-column counts,
8B-aligned dst at partition 0, fp32r-tagged producers end to end.

This walrus build accepts exactly ONE semaphore wait per instruction, so the
kernel is raw Bass: per-engine programs, counting semaphores, standalone
waits. DMA completions may reorder across transfers, so chunk DMAs chain on
per-lane semaphores (the scheme Tile's DMAHW lanes use).
"""

import functools

import numpy as np

import concourse.bass as bass
from concourse import mybir
from concourse.bass_utils import run_bass_kernel_spmd

S, B, H = 4096, 32, 512
NCORES = 8
BC = B // NCORES          # batches per core = 4
NK = H // 128             # h-chunks = 4
CH = 512                  # s-chunk width
CW = CH + 1               # chunk tile block width (1-col halo)
NCH = S // CH             # s-chunks per batch = 8
NBC = BC * NCH            # chunk-steps per core = 32
NSLOT = 4                 # enc chunk tiles (4 chunk-steps in flight)
NLANE = 4                 # DMA completion-ordering lanes
SETUP_DMAS = 2 + 4 + 1 + 4  # h_t, aff_p, 4 emb, ones, 4 slot col-seeds

F32 = mybir.dt.float32
F32R = mybir.dt.float32r


def slot(bc):
    return bc % NSLOT


@functools.lru_cache(maxsize=1)
def _build():
    nc = bass.Bass("TRN2", target_bir_lowering=False, debug=False)

    enc_t = nc.dram_tensor("enc_t", [BC, H, S], F32R, kind="ExternalInput").ap()
    m_hb = nc.dram_tensor("m_hb", [BC, H, H], F32R, kind="ExternalInput").ap()
    h_t = nc.dram_tensor("h_t", [128, NK * BC + 1], F32R,
                         kind="ExternalInput").ap()
    aff_p = nc.dram_tensor("aff_p", [128, NK * 3], F32R,
                           kind="ExternalInput").ap()
    one_v = nc.dram_tensor("one_v", [128, NK], F32R, kind="ExternalInput").ap()
    emb_a = nc.dram_tensor("emb_a", [3 * BC, S], F32, kind="ExternalInput").ap()
    out = nc.dram_tensor("out", [BC, S], F32, kind="ExternalOutput").ap()

    # SBUF (~173 KB/partition of 192)
    enc_sb = [nc.alloc_sbuf_tensor(f"enc{i}", [128, NK * CW], F32R).ap()
              for i in range(NSLOT)]
    m_sb = [[nc.alloc_sbuf_tensor(f"m{s_}_{j}", [128, H], F32R).ap()
             for j in range(NK)] for s_ in range(2)]
    ht_sb = nc.alloc_sbuf_tensor("ht", [128, NK * BC + 1], F32R).ap()
    aff_sb = nc.alloc_sbuf_tensor("aff", [128, NK * 3], F32R).ap()
    emb_sb = [nc.alloc_sbuf_tensor(f"emb{b}", [3, S], F32).ap()
              for b in range(BC)]
    p_sb = [nc.alloc_sbuf_tensor(f"p{i}", [128, NK * CH], F32R).ap()
            for i in range(2)]
    ones_sb = nc.alloc_sbuf_tensor("ones", [128, NK], F32R).ap()
    ha_sb = nc.alloc_sbuf_tensor("ha", [3, BC], F32).ap()
    e_sb = nc.alloc_sbuf_tensor("e", [128, S], F32).ap()
    e4_sb = nc.alloc_sbuf_tensor("e4", [BC, S], F32).ap()
    ex4_sb = nc.alloc_sbuf_tensor("ex4", [BC, S], F32).ap()
    o4_sb = nc.alloc_sbuf_tensor("o4", [BC, S], F32).ap()
    nmx_sb = nc.alloc_sbuf_tensor("nmx", [BC, 1], F32).ap()
    sm_sb = nc.alloc_sbuf_tensor("sm", [BC, 1], F32).ap()
    rs_sb = nc.alloc_sbuf_tensor("rs", [BC, 1], F32).ap()

    # PSUM: A region 4 banks + 2 e banks + 1 ha bank = 7 of 8
    ps_a = nc.alloc_psum_tensor("psA", [128, NK * CH], F32).ap()
    ps_e = [nc.alloc_psum_tensor(f"psE{i}", [128, CH], F32).ap()
            for i in range(2)]
    ps_ha = nc.alloc_psum_tensor("psHA", [128, CH], F32).ap()

    dma_set = nc.alloc_semaphore("dma_set")  # setup + M DMAs, serialized chain
    dma_ln = [nc.alloc_semaphore(f"dma_ln{k}") for k in range(NLANE)]
    dma_out = nc.alloc_semaphore("dma_out")
    dma_g = nc.alloc_semaphore("dma_g")      # e-row gather DMA
    pe_mm = nc.alloc_semaphore("pe_mm")      # +1 per kt MM-group (4/step)
    pe_red = nc.alloc_semaphore("pe_red")    # +1 per step reduce group
    pe_ha = nc.alloc_semaphore("pe_ha")      # +1 per batch ha-MM group
    dve_pm = nc.alloc_semaphore("dve_pm")    # +1 per P-mul pair (2/step)
    dve_aff = nc.alloc_semaphore("dve_aff")  # +1 per step (aff folded in P)
    dve_ms = nc.alloc_semaphore("dve_ms")    # +1 per DVE col-0 copy
    dve_sm = nc.alloc_semaphore("dve_sm")    # +1 reciprocal done (self-sync)
    dve_fin = nc.alloc_semaphore("dve_fin")  # +1 nmx ready, +1 final scale
    act_ha = nc.alloc_semaphore("act_ha")    # +1 per batch ha copied
    act_e = nc.alloc_semaphore("act_e")      # +1 per step e copied
    act_ex = nc.alloc_semaphore("act_ex")    # +1 exp done

    def m_thresh(b):
        return 16 * (SETUP_DMAS + 4 * (b + 1))

    with nc.Block() as blk:
        # --- SP: all DMAs (issue order fixed; per-lane completion chains) ---
        @blk.sync
        def _(sync):
            setup_srcs = [
                (ht_sb[:], h_t[:]),
                (aff_sb[:], aff_p[:]),
                *[(emb_sb[b][:], emb_a[3 * b:3 * b + 3, :])
                  for b in range(BC)],
                (ones_sb[:], one_v[:]),
                # seed col 0 of each block of each slot (c==0 chunks leave it
                # unwritten; fp32r MMs read a full even window; the value is
                # replaced in psum before use)
                *[(enc_sb[s_].rearrange("p (k w) -> p k w", k=NK)[:, :, 0:1],
                   one_v.rearrange("p (k o) -> p k o", o=1))
                  for s_ in range(NSLOT)],
            ]
            nset = 0
            with nc.allow_non_contiguous_dma(
                    reason="4x tiny one-time slot col-0 seeds (16 elems)"):
                for dst, src in setup_srcs:
                    sync.dma_start(dst, src).then_inc(dma_set, 16)
                    nset += 1
            def issue_m_set(b):
                nonlocal nset
                if b >= 2:  # WAR: batch b-2's slot free once its MMs done
                    sync.wait_ge(pe_mm, 32 * (b - 1))
                # all prior dma_set updates complete before this set issues,
                # so the per-batch full-sum threshold is order-unambiguous
                sync.wait_ge(dma_set, 16 * nset)
                for j in range(NK):
                    sync.dma_start(m_sb[b % 2][j][:],
                                   m_hb[b, j * 128:(j + 1) * 128, :]) \
                        .then_inc(dma_set, 16)
                    nset += 1
            # M sets for b=0,1 upfront; b+2's set is woven in after batch
            # b+1's chunk DMAs (its WAR wait needs batch b+1 fully issued)
            issue_m_set(0)
            issue_m_set(1)
            # chunk DMAs: ONE per step, 3D AP (p, kblock, s)
            for b in range(BC):
                for c in range(NCH):
                    # weave next-next batch's M set in mid-batch: SP is at most
                    # NSLOT steps ahead of PE here, so the WAR wait
                    # (pe_mm >= 32*b = batches < b done) is already satisfied
                    if b + 1 < BC and b >= 1 and c == NCH // 2:
                        issue_m_set(b + 1)
                    bc = b * NCH + c
                    if bc >= NSLOT:  # slot reuse: consumers of bc-4 done
                        sync.wait_ge(pe_mm, 4 * (bc - NSLOT) + 4)
                        sync.wait_ge(dve_pm, 2 * (bc - NSLOT) + 2)
                    if bc >= NLANE:  # lane chain => ordered completions
                        sync.wait_ge(dma_ln[bc % NLANE], 16 * (bc // NLANE))
                    # block kt col u holds s = c*CH - 1 + u; c==0: first real
                    # column lands at u=1 (col 0 pre-seeded)
                    s0 = c * CH - 1
                    u0, ncols = 0, CW
                    if c == 0:
                        s0, u0, ncols = 0, 1, CH
                    dst3 = enc_sb[slot(bc)].rearrange(
                        "p (k w) -> p k w", k=NK)[:, :, u0:u0 + ncols]
                    src3 = enc_t[b, :, s0:s0 + ncols].rearrange(
                        "(k p) s -> p k s", p=128)
                    sync.dma_start(dst3, src3).then_inc(dma_ln[bc % NLANE], 16)
            # gather e rows {0,32,64,96} -> contiguous [4, S] (DMA APs may
            # stride partitions; engine compute APs may not)
            sync.wait_ge(act_e, NBC)
            sync.dma_start(e4_sb[:], e_sb[0:128:32, :]).then_inc(dma_g, 16)
            sync.wait_ge(dve_fin, 2)
            sync.dma_start(out[:], o4_sb[:]).then_inc(dma_out, 16)
            sync.wait_ge(dma_out, 16)

        # --- PE ---
        @blk.tensor
        def _(tensor):
            def pe_reduce(j):
                # e_tmp[0, t] = sum_kt ones^T @ P_kt for step j
                tensor.wait_ge(dve_pm, 2 * j + 2)
                tensor.wait_ge(dve_aff, j + 1)
                if j >= 2:
                    tensor.wait_ge(act_e, j - 1)  # WAR on ps_e[j%2]
                pe_bank = ps_e[j % 2]
                for kt in range(NK):
                    mm_r = nc.tensor.matmul(
                        pe_bank[0:1, 0:CH], ones_sb[:, 0:1],
                        p_sb[j % 2][:, kt * CH:(kt + 1) * CH],
                        start=(kt == 0), stop=(kt == NK - 1))
                mm_r.then_inc(pe_red, 1)

            tensor.wait_ge(dma_set, 16 * SETUP_DMAS)  # setup inputs ready
            for b in range(BC):
                ms = m_sb[b % 2]
                tensor.wait_ge(dma_set, m_thresh(b))  # this batch's M_b ready
                # ha_b = affect^T @ h_b -> psum [3, 2] (fp32r needs even N)
                if b > 0:
                    tensor.wait_ge(act_ha, b)  # WAR on ps_ha
                for j in range(NK):
                    mm_ha = nc.tensor.matmul(
                        ps_ha[0:3, 0:2],
                        aff_sb[:, 3 * j:3 * j + 3],
                        ht_sb[:, NK * j + b:NK * j + b + 2],
                        start=(j == 0), stop=(j == NK - 1),
                    )
                mm_ha.then_inc(pe_ha, 1)
                for c in range(NCH):
                    bc = b * NCH + c
                    tensor.wait_ge(dma_ln[bc % NLANE],
                                   16 * (bc // NLANE + 1))  # chunk tile in
                    for kt in range(NK):
                        g = 4 * bc + kt
                        if g >= 4:  # WAR on psA bank kt: P-mul pair done
                            gp = g - 4
                            tensor.wait_ge(dve_pm,
                                           2 * (gp // 4) + (gp % 4) // 2 + 1)
                        for j in range(NK):
                            mm = nc.tensor.matmul(
                                ps_a[:, kt * CH:(kt + 1) * CH],
                                ms[j][:, kt * 128:(kt + 1) * 128],
                                enc_sb[slot(bc)][:, j * CW:j * CW + CH],
                                start=(j == 0), stop=(j == NK - 1),
                            )
                        mm.then_inc(pe_mm, 1)
                    # deferred reduce of the PREVIOUS step: its DVE pairs and
                    # aff finished during this step's MM groups -> no PE stall
                    if bc >= 1:
                        pe_reduce(bc - 1)
            pe_reduce(NBC - 1)

        # --- DVE ---
        @blk.vector
        def _(vector):
            n_ms = 0
            for b in range(BC):
                for c in range(NCH):
                    bc = b * NCH + c
                    if bc >= 2:
                        vector.wait_ge(pe_red, bc - 1)  # WAR on p[bc%2]
                    for half in range(2):  # P-mul banks (0,1) then (2,3)
                        vector.wait_ge(pe_mm, 4 * bc + 2 * half + 2)
                        if c == 0:
                            # psum col 0 of each bank := h_k (A'[-1] = h)
                            for kt in (2 * half, 2 * half + 1):
                                nc.vector.tensor_copy(
                                    ps_a[:, kt * CH:kt * CH + 1],
                                    ht_sb[:, NK * kt + b:NK * kt + b + 1]) \
                                    .then_inc(dve_ms, 1)
                                n_ms += 1
                            vector.wait_ge(dve_ms, n_ms)
                        pa3 = ps_a.rearrange("p (k s) -> p k s", k=NK)[
                            :, 2 * half:2 * half + 2, :]
                        en3 = enc_sb[slot(bc)].rearrange(
                            "p (k w) -> p k w", k=NK)[
                            :, 2 * half:2 * half + 2, 1:CW]
                        po3 = p_sb[bc % 2].rearrange(
                            "p (k s) -> p k s", k=NK)[
                            :, 2 * half:2 * half + 2, :]
                        nc.vector.tensor_mul(po3, pa3, en3) \
                            .then_inc(dve_pm, 1)
                    # affect term into P rows 0..2 (after pair 0 completes)
                    vector.wait_ge(act_ha, b + 1)   # ha_sb[., b] ready
                    vector.wait_ge(dve_pm, 2 * bc + 1)
                    nc.vector.scalar_tensor_tensor(
                        p_sb[bc % 2][0:3, 0:CH],
                        emb_sb[b][0:3, c * CH:(c + 1) * CH],
                        ha_sb[0:3, b:b + 1],
                        p_sb[bc % 2][0:3, 0:CH],
                        mybir.AluOpType.mult, mybir.AluOpType.add,
                    ).then_inc(dve_aff, 1)
            # softmax (batched on contiguous [4, S])
            vector.wait_ge(dma_g, 16)
            nc.vector.tensor_reduce(nmx_sb[:], e4_sb[:], mybir.AxisListType.X,
                                    mybir.AluOpType.max, negate=True) \
                .then_inc(dve_fin, 1)   # "nmx ready" (ACT waits 1)
            vector.wait_ge(act_ex, 1)
            nc.vector.reciprocal(rs_sb[:], sm_sb[:]).then_inc(dve_sm, 1)
            vector.wait_ge(dve_sm, 1)
            nc.vector.tensor_scalar_mul(o4_sb[:], ex4_sb[:], rs_sb[0:BC, 0:1]) \
                .then_inc(dve_fin, 1)   # dve_fin==2 -> SP may DMA out

        # --- ACT: PSUM->SBUF copies, exp ---
        @blk.scalar
        def _(scalar):
            for b in range(BC):
                scalar.wait_ge(pe_ha, b + 1)
                nc.scalar.copy(ha_sb[0:3, b:b + 1], ps_ha[0:3, 0:1]) \
                    .then_inc(act_ha, 1)
                for c in range(NCH):
                    bc = b * NCH + c
                    scalar.wait_ge(pe_red, bc + 1)
                    nc.scalar.copy(e_sb[32 * b:32 * b + 1, c * CH:(c + 1) * CH],
                                   ps_e[bc % 2][0:1, 0:CH]).then_inc(act_e, 1)
            scalar.wait_ge(dve_fin, 1)  # nmx ready
            nc.scalar.activation(ex4_sb[:], e4_sb[:],
                                 mybir.ActivationFunctionType.Exp,
                                 bias=nmx_sb[0:BC, 0:1],
                                 accum_out=sm_sb[0:BC, 0:1]) \
                .then_inc(act_ex, 1)

    # no end-of-program sem clears: each PJRT execution starts with fresh
    # semaphore state (verified: 3 back-to-back executions of one loaded NEFF
    # each gave correct, input-scaled results).
    return nc


def _shard_host(hidden, encoder_outputs, embedding, bigram_matrix, affect_matrix):
    """Build per-core input maps. Only layout/scaling prep happens here."""
    h = np.asarray(hidden, dtype=np.float32)[0]              # [B, H]
    enc = np.asarray(encoder_outputs, dtype=np.float32)      # [S, B, H]
    emb = np.asarray(embedding, dtype=np.float32)            # [S, B, 3]
    m = np.ascontiguousarray(np.asarray(bigram_matrix, dtype=np.float32))
    aff = np.asarray(affect_matrix, dtype=np.float32)        # [H, 3]

    enc_bhs = np.ascontiguousarray(enc.transpose(1, 2, 0))   # [B, H, S]
    emb_bks = np.ascontiguousarray(emb.transpose(1, 2, 0))   # [B, 3, S]
    aff_pk = np.ascontiguousarray(
        aff.reshape(NK, 128, 3).transpose(1, 0, 2).reshape(128, NK * 3))
    # h folded into M per batch: m_hb[b, j, k] = M[j, k] * h[b, k]
    m_hb_all = np.ascontiguousarray(m[None, :, :] * h[:, None, :])  # [B,H,H]

    in_maps = []
    for co in range(NCORES):
        b0 = co * BC
        h_sl = h[b0:b0 + BC]                                  # [BC, H]
        ht_pk = np.concatenate([
            h_sl.reshape(BC, NK, 128).transpose(2, 1, 0).reshape(128, NK * BC),
            np.zeros((128, 1), dtype=np.float32)], axis=1)
        in_maps.append({
            "enc_t": enc_bhs[b0:b0 + BC],                     # [BC, H, S]
            "m_hb": m_hb_all[b0:b0 + BC],
            "h_t": np.ascontiguousarray(ht_pk),
            "aff_p": aff_pk,
            "one_v": np.ones((128, NK), dtype=np.float32),
            "emb_a": emb_bks[b0:b0 + BC].reshape(3 * BC, S),
        })
    return in_maps


def kernel(hidden, encoder_outputs, embedding, bigram_matrix, affect_matrix,
           _want_results=False, _spmd_kwargs=None):
    nc = _build()
    in_maps = _shard_host(hidden, encoder_outputs, embedding,
                          bigram_matrix, affect_matrix)
    res = run_bass_kernel_spmd(nc, in_maps, core_ids=list(range(NCORES)),
                               **(_spmd_kwargs or {}))
    outp = np.empty((B, 1, S), dtype=np.float32)
    for co in range(NCORES):
        outp[co * BC:(co + 1) * BC, 0, :] = res.results[co]["out"]
    if _want_results:
        return outp, res
    return outp

